# revision 2
# baseline (speedup 1.0000x reference)
"""Trimmed-MAE loss (MAETrimLoss) Bass kernel for Trainium2, 8 NeuronCores. v8.

Math: per image, loss_sum = sum of the K smallest |p-t| values (K = 0.8*M,
M = H*W). With T0 = the 0.8-quantile of |N(0,sqrt(2))| = 1.8124, the
threshold identity R(T) = sum min(|d|,T) - (M-K)*T equals loss_sum at T = the
sample quantile t*; R is flat at its max (sample-quantile noise is +-0.003
for M=307200), so the fixed T0 matches loss_sum to ~1e-5 rel for any
randn-like input. On device, per image:
  sum min(|d|,T0) = sum|d| + M*T0 - sum max(|d|,T0).

Measured TRN2 constraints this design is built around (each ~15min of HW
bisection; do not regress):
- ACT accum_out costs ~0.4us PER INSTRUCTION (8 of them = +3.2us), so the
  drains carry no accum.
- An accum-bearing DVE tensor_scalar streams at ~1 elem/cycle/lane
  effective (2x-mode + mandatory pipe-drain): ONE full sweep of the data
  (~9.5us) is the entire DVE budget. Kernels with two DVE sweeps (any
  combination of scalars, out tiles, granularities, chaining, DMA rings)
  measure 20-25us. op0=min is ~4x slower again (no fast uop). DVE reads of
  PSUM (incl. custom DVE ops, which are 1x-only here) cost ~2.4us/pass.
- So: sum max(|d|,T0) is the single DVE sweep (the one proven-fast shape:
  tensor_scalar(max, T0, add-accum), bf16 SBUF per half), and the sum|d|
  tax is SPLIT THREE WAYS: images 0,1 carry it as ACT accum_out on their
  drains (+4 x 0.4us on ACT), images 2,3 on the PE via 3 accumulating
  ones-matmuls per half (bf16 column-select stationary: block i has only
  column i set, so image i's column sums land in row i of one [128,512]
  PSUM bank; matmul PSUM writes must fit one bank; +12 matmuls ~2.7us on
  PE). All-on-ACT measured 12.55us, all-on-PE 12.85us; the split puts
  every engine at ~9-10us. psE rows 2..3 are ACT-copied to SBUF and DMA'd
  out; the host sums 512 f32 per image.

Inputs are quantized to fp8e4m3 on the host (harness gate is 2e-2 relative;
fp8 contributes ~1e-3): 4x less HBM traffic than fp32 -> 2.46 MB/core.
- TensorE subtracts via DoubleRow fp8 identity matmuls: stationary [I | -I]
  (3D AP [128,2,128]), moving [p_h | t_h] -> PSUM f32 d, 512-col chunks.
- All 8 halves drain via ACT activation(Abs) psum->bf16 e.
- DMA: 4 transfers of [128, 4800B] per core, alternating SP/Pool rings.
Sharding: pure data parallel, 4 images x 8 cores; host combines in f64 and
falls back to an exact host computation if the 0.8-quantile of |p-t| sits
far from T0 (distribution-shift guard).
"""

import numpy as np
import ml_dtypes

import concourse.bacc as bacc
import concourse.mybir as mybir
from concourse.tile import TileContext
from concourse.bass_utils import run_bass_kernel_spmd

B, C, H, W = 32, 1, 480, 640
M = H * W                      # 307200 elements per image
K = int(0.8 * M)               # 245760
N_CORES = 8
IMGS = B // N_CORES            # 4
P = 128
FI = M // P                    # 2400 image cols at 128 partitions
HALF = FI // 2                 # 1200
QH = 512                       # psE width = one PSUM bank
T0 = float(np.float32(1.8124))
FP8 = ml_dtypes.float8_e4m3
BF16 = ml_dtypes.bfloat16
XW = 2 * FI                    # 4800
NCOL = 24
NCHUNK = 4

DMA_ORDER = [(3, 0), (0, 0), (3, 1), (1, 0), (0, 1), (1, 1), (2, 0), (2, 1)]
MM_CHUNKS = [(0, 512), (512, 512), (1024, 176)]
ACC_IMGS = (0, 1)   # images whose sum|d| rides ACT accum (rest: PE ones-mm)

_CACHE = {}


def build_nc(repeats: int = 1):
    nc = bacc.Bacc()
    f32 = mybir.dt.float32
    f8 = mybir.dt.float8e4
    bf16 = mybir.dt.bfloat16
    A = mybir.AluOpType
    ABS = mybir.ActivationFunctionType.Abs

    x_in = nc.declare_dram_parameter("x", [P, IMGS * XW], f8, isOutput=False)
    w_in = nc.declare_dram_parameter("w", [P, 256], f8, isOutput=False)
    ob_in = nc.declare_dram_parameter("ob", [P, IMGS * P], bf16, isOutput=False)
    out = nc.declare_dram_parameter("acc", [P, NCOL], f32, isOutput=True)
    s_out = nc.declare_dram_parameter("s", [IMGS, QH], f32, isOutput=True)

    zb_ap = nc.const_aps.aps[(f32, 0.0)]
    zscr = nc.alloc_sbuf_tensor("zscr", [P, 1], f32)
    # dummy activation: pulls the ACT function table load off the hot path
    nc.scalar.activation(zscr.ap(), zb_ap, ABS, bias=zb_ap, scale=1.0)

    ones_halves = [(i, h) for (i, h) in DMA_ORDER if i not in ACC_IMGS]
    n_mm = {ih: (k == 0, k == len(ones_halves) - 1)
            for k, ih in enumerate(ones_halves)}

    with TileContext(nc) as tc:
        with tc.tile_pool(name="data", bufs=3) as dpool, \
             tc.tile_pool(name="big", bufs=2, space="SBUF") as bpool, \
             tc.tile_pool(name="ps", bufs=2, space="PSUM") as pspool, \
             tc.tile_pool(name="pse", bufs=1, space="PSUM") as psepool, \
             tc.tile_pool(name="accp", bufs=2) as apool:
            w_t = dpool.tile([P, 256], f8, tag="w")
            ob_t = dpool.tile([P, IMGS * P], bf16, tag="ob")
            nc.sync.dma_start(out=w_t[:], in_=w_in.ap())
            nc.sync.dma_start(out=ob_t[:], in_=ob_in.ap())
            lhsT2w = w_t[:].rearrange("p (two f) -> p two f", two=2)
            for _ in range(repeats):
                acc = apool.tile([P, NCOL], f32, tag="acc")
                nc.vector.memset(acc[:], 0.0)

                xt = {}
                seg = IMGS * XW // NCHUNK
                per_chunk = len(DMA_ORDER) // NCHUNK
                for ci in range(NCHUNK):
                    t = dpool.tile([P, seg], f8, tag=f"xc{ci}", name=f"xc{ci}")
                    ring = nc.sync if ci % 2 == 0 else nc.gpsimd
                    ring.dma_start(
                        out=t[:], in_=x_in.ap()[:, ci * seg:(ci + 1) * seg])
                    for j in range(per_chunk):
                        i, h = DMA_ORDER[ci * per_chunk + j]
                        xt[(i, h)] = (t, j * 2 * HALF)

                d = {}
                for i in range(IMGS):
                    d[i] = bpool.tile([P, FI], bf16, tag=f"d{i}", name=f"d{i}")
                bscr = bpool.tile([P, FI], bf16, tag="bscr", bufs=1)
                psE = psepool.tile([P, QH], f32, tag="psE", name="psE")

                def emit_ones_mm(ih):
                    i, h = ih
                    first, last = n_mm[ih]
                    st = ob_t[:, i * P:(i + 1) * P]
                    e_half = d[i][:, h * HALF:(h + 1) * HALF]
                    for ci, (c0, cw) in enumerate(MM_CHUNKS):
                        nc.tensor.matmul(
                            psE[:, 0:cw], st, e_half[:, c0:c0 + cw],
                            start=(first and ci == 0),
                            stop=(last and ci == len(MM_CHUNKS) - 1))

                pending = []
                for oi, (i, h) in enumerate(DMA_ORDER):
                    xtile, xoff = xt[(i, h)]
                    ps = pspool.tile([P, 1536], f32, tag="psH", bufs=2,
                                     name="psH")
                    rhs2 = xtile[:, xoff:xoff + 2 * HALF].rearrange(
                        "p (two f) -> p two f", two=2)
                    for (c0, cw) in MM_CHUNKS:
                        nc.tensor.matmul(ps[:, c0:c0 + cw], lhsT2w,
                                         rhs2[:, :, c0:c0 + cw],
                                         start=True, stop=True,
                                         perf_mode=mybir.MatmulPerfMode.DoubleRow)
                    dst = d[i][:, h * HALF:(h + 1) * HALF]
                    nc.scalar.activation(
                        dst, ps[:, 0:HALF], ABS, bias=zb_ap, scale=1.0,
                        accum_out=(acc[:, oi:oi + 1] if i in ACC_IMGS
                                   else None))
                    nc.vector.tensor_scalar(
                        bscr[:, h * HALF:(h + 1) * HALF], dst,
                        float(T0), None, A.max, A.add,
                        accum_out=acc[:, 8 + oi:9 + oi])
                    if i not in ACC_IMGS:
                        pending.append((i, h))
                    # ones-matmuls lag one half behind so the in-order PE
                    # stream never waits on an ACT drain still in flight
                    if len(pending) > 1:
                        emit_ones_mm(pending.pop(0))
                for ih in pending:
                    emit_ones_mm(ih)

                s_t = apool.tile([IMGS, QH], f32, tag="s_t")
                nc.scalar.copy(s_t[:], psE[0:IMGS, :])
                nc.sync.dma_start(out=s_out.ap(), in_=s_t[:])
                nc.sync.dma_start(out=out.ap(), in_=acc[:])
    nc.finalize()
    return nc


def _get_nc():
    if "nc" not in _CACHE:
        _CACHE["nc"] = build_nc()
    return _CACHE["nc"]


def make_w():
    wm = np.zeros((P, 256), dtype=np.float32)
    wm[:, 0:128] = np.eye(P)
    wm[:, 128:256] = -np.eye(P)
    return wm.astype(FP8)


def make_ones_bf16():
    """[P, IMGS*P] bf16: stationary block i has only column i set to ones."""
    ob = np.zeros((P, IMGS * P), dtype=BF16)
    for i in range(IMGS):
        ob[:, i * P + i] = 1.0
    return ob


def shard_inputs(prediction, target):
    """fp8-quantize, half-interleave, core-partition-major layout.

    Returns x [N_CORES, P, IMGS*XW]: per core one contiguous per-partition
    stream of [p_h | t_h] blocks in DMA_ORDER sequence (long DMA lines).
    """
    pr = np.clip(prediction.reshape(B, P, FI), -200.0, 200.0).astype(FP8)
    tr = np.clip(target.reshape(B, P, FI), -200.0, 200.0).astype(FP8)
    x = np.empty((N_CORES, P, IMGS * XW), dtype=FP8)
    for k, (i, h) in enumerate(DMA_ORDER):
        o = 2 * HALF * k
        for c in range(N_CORES):
            b = c * IMGS + i
            x[c, :, o:o + HALF] = pr[b, :, h * HALF:(h + 1) * HALF]
            x[c, :, o + HALF:o + 2 * HALF] = tr[b, :, h * HALF:(h + 1) * HALF]
    return x


def combine(acc_results, s_results):
    """Per-core [P,NCOL] max-accums + [IMGS,QH] psE rows -> losses (f64)."""
    T0d = float(T0)
    n = len(acc_results)
    losses = np.empty(n * IMGS)
    for c in range(n):
        a = acc_results[c].astype(np.float64)
        s = s_results[c].astype(np.float64)
        for i in range(IMGS):
            ois = [oi for oi, (ii, hh) in enumerate(DMA_ORDER) if ii == i]
            s_max = sum(a[:, 8 + oi].sum() for oi in ois)
            if i in ACC_IMGS:
                s_e = sum(a[:, oi].sum() for oi in ois)
            else:
                s_e = s[i].sum()
            s_min = s_e + M * T0d - s_max
            losses[c * IMGS + i] = (s_min - (M - K) * T0d) / (2.0 * M)
    return losses


def kernel(prediction, target, mask):
    prediction = np.asarray(prediction, dtype=np.float32)
    target = np.asarray(target, dtype=np.float32)
    nc = _get_nc()
    x = shard_inputs(prediction, target)
    wq = make_w()
    ob = make_ones_bf16()
    in_maps = [{"x": x[c], "w": wq, "ob": ob} for c in range(N_CORES)]
    res = run_bass_kernel_spmd(nc, in_maps, core_ids=list(range(N_CORES)))
    losses = combine([res.results[c]["acc"] for c in range(N_CORES)],
                     [res.results[c]["s"] for c in range(N_CORES)])

    # safety: check the 0.8-quantile of |p-t| sits in the flat window via a
    # subsample; exact host fallback for any image where it does not.
    rng = np.random.default_rng(12345)
    idx = rng.integers(0, M, size=4096)
    dsub = np.abs(prediction.reshape(B, M)[:, idx].astype(np.float64)
                  - target.reshape(B, M)[:, idx].astype(np.float64))
    q = np.quantile(dsub, 0.8, axis=1)
    bad = np.abs(q - T0) > 0.12
    if bad.any():
        a = np.abs(prediction.reshape(B, -1)[bad].astype(np.float64) -
                   target.reshape(B, -1)[bad].astype(np.float64))
        part = np.partition(a, K - 1, axis=1)
        t_ex = part[:, K - 1]
        below = np.where(a < t_ex[:, None], a, 0.0)
        cnt = (a < t_ex[:, None]).sum(axis=1)
        losses[bad] = (below.sum(axis=1) + (K - cnt) * t_ex) / (2 * M)
    return np.asarray(np.float32(np.mean(losses)))


# revision 4
# speedup vs baseline: 1.0032x; 1.0032x over previous
"""Trimmed-MAE loss (MAETrimLoss) Bass kernel for Trainium2, 8 NeuronCores. v8.

Math: per image, loss_sum = sum of the K smallest |p-t| values (K = 0.8*M,
M = H*W). With T0 = the 0.8-quantile of |N(0,sqrt(2))| = 1.8124, the
threshold identity R(T) = sum min(|d|,T) - (M-K)*T equals loss_sum at T = the
sample quantile t*; R is flat at its max (sample-quantile noise is +-0.003
for M=307200), so the fixed T0 matches loss_sum to ~1e-5 rel for any
randn-like input. On device, per image:
  sum min(|d|,T0) = sum|d| + M*T0 - sum max(|d|,T0).

Measured TRN2 constraints this design is built around (each ~15min of HW
bisection; do not regress):
- ACT accum_out costs ~0.4us PER INSTRUCTION (8 of them = +3.2us), so the
  drains carry no accum.
- An accum-bearing DVE tensor_scalar streams at ~1 elem/cycle/lane
  effective (2x-mode + mandatory pipe-drain): ONE full sweep of the data
  (~9.5us) is the entire DVE budget. Kernels with two DVE sweeps (any
  combination of scalars, out tiles, granularities, chaining, DMA rings)
  measure 20-25us. op0=min is ~4x slower again (no fast uop). DVE reads of
  PSUM (incl. custom DVE ops, which are 1x-only here) cost ~2.4us/pass.
- So: sum max(|d|,T0) is the single DVE sweep (the one proven-fast shape:
  tensor_scalar(max, T0, add-accum), bf16 SBUF per half), and the sum|d|
  tax is SPLIT THREE WAYS: images 0,1 carry it as ACT accum_out on their
  drains (+4 x 0.4us on ACT), images 2,3 on the PE via 3 accumulating
  ones-matmuls per half (bf16 column-select stationary: block i has only
  column i set, so image i's column sums land in row i of one [128,512]
  PSUM bank; matmul PSUM writes must fit one bank; +12 matmuls ~2.7us on
  PE). All-on-ACT measured 12.55us, all-on-PE 12.85us; the split puts
  every engine at ~9-10us. psE rows 2..3 are ACT-copied to SBUF and DMA'd
  out; the host sums 512 f32 per image.

Inputs are quantized to fp8e4m3 on the host (harness gate is 2e-2 relative;
fp8 contributes ~1e-3): 4x less HBM traffic than fp32 -> 2.46 MB/core.
- TensorE subtracts via DoubleRow fp8 identity matmuls: stationary [I | -I]
  (3D AP [128,2,128]), moving [p_h | t_h] -> PSUM f32 d, 512-col chunks.
- All 8 halves drain via ACT activation(Abs) psum->bf16 e.
- DMA: 4 transfers of [128, 4800B] per core, alternating SP/Pool rings.
Sharding: pure data parallel, 4 images x 8 cores; host combines in f64 and
falls back to an exact host computation if the 0.8-quantile of |p-t| sits
far from T0 (distribution-shift guard).
"""

import numpy as np
import ml_dtypes

import concourse.bacc as bacc
import concourse.mybir as mybir
from concourse.tile import TileContext
from concourse.bass_utils import run_bass_kernel_spmd

B, C, H, W = 32, 1, 480, 640
M = H * W                      # 307200 elements per image
K = int(0.8 * M)               # 245760
N_CORES = 8
IMGS = B // N_CORES            # 4
P = 128
FI = M // P                    # 2400 image cols at 128 partitions
HALF = FI // 2                 # 1200
QH = 512                       # psE width = one PSUM bank
T0 = float(np.float32(1.8124))
FP8 = ml_dtypes.float8_e4m3
BF16 = ml_dtypes.bfloat16
XW = 2 * FI                    # 4800
NCOL = 24
NCHUNK = 4

DMA_ORDER = [(3, 0), (0, 0), (3, 1), (1, 0), (0, 1), (1, 1), (2, 0), (2, 1)]
MM_CHUNKS = [(0, 512), (512, 512), (1024, 176)]
ACC_IMGS = (0, 1)   # images whose sum|d| rides ACT accum (rest: PE ones-mm)

_CACHE = {}


def build_nc(repeats: int = 1):
    nc = bacc.Bacc()
    f32 = mybir.dt.float32
    f8 = mybir.dt.float8e4
    bf16 = mybir.dt.bfloat16
    A = mybir.AluOpType
    ABS = mybir.ActivationFunctionType.Abs

    x_in = nc.declare_dram_parameter("x", [P, IMGS * XW], f8, isOutput=False)
    w_in = nc.declare_dram_parameter("w", [P, 256], f8, isOutput=False)
    ob_in = nc.declare_dram_parameter("ob", [P, IMGS * P], bf16, isOutput=False)
    out = nc.declare_dram_parameter("acc", [P, NCOL], f32, isOutput=True)
    s_out = nc.declare_dram_parameter("s", [IMGS, QH], f32, isOutput=True)

    zb_ap = nc.const_aps.aps[(f32, 0.0)]
    zscr = nc.alloc_sbuf_tensor("zscr", [P, 1], f32)
    # dummy activation: pulls the ACT function table load off the hot path
    nc.scalar.activation(zscr.ap(), zb_ap, ABS, bias=zb_ap, scale=1.0)

    ones_halves = [(i, h) for (i, h) in DMA_ORDER if i not in ACC_IMGS]
    n_mm = {ih: (k == 0, k == len(ones_halves) - 1)
            for k, ih in enumerate(ones_halves)}

    with TileContext(nc) as tc:
        with tc.tile_pool(name="data", bufs=3) as dpool, \
             tc.tile_pool(name="big", bufs=2, space="SBUF") as bpool, \
             tc.tile_pool(name="ps", bufs=2, space="PSUM") as pspool, \
             tc.tile_pool(name="pse", bufs=1, space="PSUM") as psepool, \
             tc.tile_pool(name="accp", bufs=2) as apool:
            w_t = dpool.tile([P, 256], f8, tag="w")
            ob_t = dpool.tile([P, IMGS * P], bf16, tag="ob")
            nc.sync.dma_start(out=w_t[:], in_=w_in.ap())
            nc.sync.dma_start(out=ob_t[:], in_=ob_in.ap())
            lhsT2w = w_t[:].rearrange("p (two f) -> p two f", two=2)
            for _ in range(repeats):
                acc = apool.tile([P, NCOL], f32, tag="acc")
                nc.vector.memset(acc[:], 0.0)

                xt = {}
                seg = IMGS * XW // NCHUNK
                per_chunk = len(DMA_ORDER) // NCHUNK
                for ci in range(NCHUNK):
                    t = dpool.tile([P, seg], f8, tag=f"xc{ci}", name=f"xc{ci}")
                    ring = nc.sync if ci % 2 == 0 else nc.gpsimd
                    ring.dma_start(
                        out=t[:], in_=x_in.ap()[:, ci * seg:(ci + 1) * seg])
                    for j in range(per_chunk):
                        i, h = DMA_ORDER[ci * per_chunk + j]
                        xt[(i, h)] = (t, j * 2 * HALF)

                d = {}
                for i in range(IMGS):
                    d[i] = bpool.tile([P, FI], bf16, tag=f"d{i}", name=f"d{i}")
                bscr = bpool.tile([P, FI], bf16, tag="bscr", bufs=1)
                psE = psepool.tile([P, QH], f32, tag="psE", name="psE")

                def emit_ones_mm(ih):
                    i, h = ih
                    first, last = n_mm[ih]
                    st = ob_t[:, i * P:(i + 1) * P]
                    e_half = d[i][:, h * HALF:(h + 1) * HALF]
                    for ci, (c0, cw) in enumerate(MM_CHUNKS):
                        nc.tensor.matmul(
                            psE[:, 0:cw], st, e_half[:, c0:c0 + cw],
                            start=(first and ci == 0),
                            stop=(last and ci == len(MM_CHUNKS) - 1))

                pending = []
                for oi, (i, h) in enumerate(DMA_ORDER):
                    xtile, xoff = xt[(i, h)]
                    ps = pspool.tile([P, 1536], f32, tag="psH", bufs=2,
                                     name="psH")
                    rhs2 = xtile[:, xoff:xoff + 2 * HALF].rearrange(
                        "p (two f) -> p two f", two=2)
                    for (c0, cw) in MM_CHUNKS:
                        nc.tensor.matmul(ps[:, c0:c0 + cw], lhsT2w,
                                         rhs2[:, :, c0:c0 + cw],
                                         start=True, stop=True,
                                         perf_mode=mybir.MatmulPerfMode.DoubleRow)
                    dst = d[i][:, h * HALF:(h + 1) * HALF]
                    nc.scalar.activation(
                        dst, ps[:, 0:HALF], ABS, bias=zb_ap, scale=1.0,
                        accum_out=(acc[:, oi:oi + 1] if i in ACC_IMGS
                                   else None))
                    nc.vector.tensor_scalar(
                        bscr[:, h * HALF:(h + 1) * HALF], dst,
                        float(T0), None, A.max, A.add,
                        accum_out=acc[:, 8 + oi:9 + oi])
                    if i not in ACC_IMGS:
                        pending.append((i, h))
                    # ones-matmuls lag one half behind so the in-order PE
                    # stream never waits on an ACT drain still in flight
                    if len(pending) > 1:
                        emit_ones_mm(pending.pop(0))
                for ih in pending:
                    emit_ones_mm(ih)

                s_t = apool.tile([IMGS, QH], f32, tag="s_t")
                nc.scalar.copy(s_t[:], psE[0:IMGS, :])
                # s rides the SWDGE (gpsimd) ring: a 4th per-iteration
                # transfer on the sync HWDGE ring made per-iteration time
                # GROW with repeat count (11.0us at R=33/129 -> 13.3us at
                # R=65/257); keeping sync at the baseline-proven 3
                # transfers/iteration avoids the backlog.
                nc.gpsimd.dma_start(out=s_out.ap(), in_=s_t[:])
                nc.sync.dma_start(out=out.ap(), in_=acc[:])
    nc.finalize()
    return nc


def _get_nc():
    if "nc" not in _CACHE:
        _CACHE["nc"] = build_nc()
    return _CACHE["nc"]


def make_w():
    wm = np.zeros((P, 256), dtype=np.float32)
    wm[:, 0:128] = np.eye(P)
    wm[:, 128:256] = -np.eye(P)
    return wm.astype(FP8)


def make_ones_bf16():
    """[P, IMGS*P] bf16: stationary block i has only column i set to ones."""
    ob = np.zeros((P, IMGS * P), dtype=BF16)
    for i in range(IMGS):
        ob[:, i * P + i] = 1.0
    return ob


def shard_inputs(prediction, target):
    """fp8-quantize, half-interleave, core-partition-major layout.

    Returns x [N_CORES, P, IMGS*XW]: per core one contiguous per-partition
    stream of [p_h | t_h] blocks in DMA_ORDER sequence (long DMA lines).
    """
    pr = np.clip(prediction.reshape(B, P, FI), -200.0, 200.0).astype(FP8)
    tr = np.clip(target.reshape(B, P, FI), -200.0, 200.0).astype(FP8)
    x = np.empty((N_CORES, P, IMGS * XW), dtype=FP8)
    for k, (i, h) in enumerate(DMA_ORDER):
        o = 2 * HALF * k
        for c in range(N_CORES):
            b = c * IMGS + i
            x[c, :, o:o + HALF] = pr[b, :, h * HALF:(h + 1) * HALF]
            x[c, :, o + HALF:o + 2 * HALF] = tr[b, :, h * HALF:(h + 1) * HALF]
    return x


def combine(acc_results, s_results):
    """Per-core [P,NCOL] max-accums + [IMGS,QH] psE rows -> losses (f64)."""
    T0d = float(T0)
    n = len(acc_results)
    losses = np.empty(n * IMGS)
    for c in range(n):
        a = acc_results[c].astype(np.float64)
        s = s_results[c].astype(np.float64)
        for i in range(IMGS):
            ois = [oi for oi, (ii, hh) in enumerate(DMA_ORDER) if ii == i]
            s_max = sum(a[:, 8 + oi].sum() for oi in ois)
            if i in ACC_IMGS:
                s_e = sum(a[:, oi].sum() for oi in ois)
            else:
                s_e = s[i].sum()
            s_min = s_e + M * T0d - s_max
            losses[c * IMGS + i] = (s_min - (M - K) * T0d) / (2.0 * M)
    return losses


def kernel(prediction, target, mask):
    prediction = np.asarray(prediction, dtype=np.float32)
    target = np.asarray(target, dtype=np.float32)
    nc = _get_nc()
    x = shard_inputs(prediction, target)
    wq = make_w()
    ob = make_ones_bf16()
    in_maps = [{"x": x[c], "w": wq, "ob": ob} for c in range(N_CORES)]
    res = run_bass_kernel_spmd(nc, in_maps, core_ids=list(range(N_CORES)))
    losses = combine([res.results[c]["acc"] for c in range(N_CORES)],
                     [res.results[c]["s"] for c in range(N_CORES)])

    # safety: check the 0.8-quantile of |p-t| sits in the flat window via a
    # subsample; exact host fallback for any image where it does not.
    rng = np.random.default_rng(12345)
    idx = rng.integers(0, M, size=4096)
    dsub = np.abs(prediction.reshape(B, M)[:, idx].astype(np.float64)
                  - target.reshape(B, M)[:, idx].astype(np.float64))
    q = np.quantile(dsub, 0.8, axis=1)
    bad = np.abs(q - T0) > 0.12
    if bad.any():
        a = np.abs(prediction.reshape(B, -1)[bad].astype(np.float64) -
                   target.reshape(B, -1)[bad].astype(np.float64))
        part = np.partition(a, K - 1, axis=1)
        t_ex = part[:, K - 1]
        below = np.where(a < t_ex[:, None], a, 0.0)
        cnt = (a < t_ex[:, None]).sum(axis=1)
        losses[bad] = (below.sum(axis=1) + (K - cnt) * t_ex) / (2 * M)
    return np.asarray(np.float32(np.mean(losses)))
